# revision 15
# baseline (speedup 1.0000x reference)
"""Distributed attention kernel for trn2 (8 NeuronCores).

Problem: B=16, S=4096, D=64 attention, out = softmax(Q K^T / sqrt(D)) V.
Sharding: batch dim B across 8 cores (2 batches per core), no collectives.

ScalarE exp is the bottleneck engine (33.5M exps/core, 1 elem/cyc/lane
@1.2GHz = 218us floor). This version widens each ACTIVATE to a 3-bank
[128, 1536] f32 PSUM access pattern, amortizing the 222-cycle fixed
overhead over 1536 elements (vs 1024 before): ACT busy ~250us vs 263us.
PSUM budget (8 banks): 2x sp[128,1536] (6) + ot[128,512] (1) +
rs[128,512] (1) -- zero banks left for anything else, so:

  - Unit = 512 scores/partition (one 128-row k chunk x one 512-wide
    q tile), ONE matmul: lhsT = ktp[64h:64h+64, 128 k-cols] (128-col
    stationary -> full-bank [128,512] f32 out), rhs = 64 partitions of
    the duplicated Q^T. "k halves" are even/odd 128-row chunks (a
    [128,128] transpose of a [rows, 2x64d] natural slab stacks two
    chunks' K^T on partition halves). sp tiles hold 3 units; pipeline:
    exp(tile t) -> S(tile t+2, same buffer) -> AV/rs for tile t.
  - AV: 2 col-tiled 64-col matmuls per chunk pair accumulate even/odd
    chunk sums into ot halves over a q tile's 16 pairs; rs accumulates
    sum(exp) via a [128,32] ones stationary at 4 tile_positions (rows
    hold 4 quarter-sums, folded in phase C by a 4-hot wsel matmul).
  - Phase C: ot/rs CASTs to SBUF (DVE), then (deferred 2 units into the
    next q tile, hosted in the rs bank between rs accumulations) a
    stacked-identity matmul folds+transposes ot, a wsel matmul folds the
    denominators; DVE reciprocal+scale; output DMA on the sync queue.
  - Q^T/K^T transposes: batch 0 on the PE (24 in the head via two sp
    scratch tiles allocated before the S stream, 24 more in per-q-tile
    rs-bank windows of 6); batch 1 (far deadlines) on the DMA-xbar ring
    (nc.sync dma_start(transpose=True) -> staging tile -> Pool-engine
    copy). Queue discipline matters: deps are encoded as per-engine
    completion counters, so any slow producer poisons every later op on
    its queue -- the DVE stream must stay prompt (phase C only), the
    ~1.2us-serialized xbar ring gets only far-deadline work, and output
    DMAs ride sync where lateness is harmless (fin pool bufs=6 absorbs
    the WAR).
"""

import os

import numpy as np

import concourse.bass as bass
import concourse.mybir as mybir
from concourse import bacc
from concourse.tile import TileContext
from concourse.bass_utils import run_bass_kernel_spmd

DEBUG_DUMP = bool(os.environ.get("KERNEL_DEBUG_DUMP"))

B, S, D = 16, 4096, 64
N_CORES = 8
BS = B // N_CORES  # batches per core
SCALE = 1.0 / np.sqrt(D)  # 0.125
F32 = mybir.dt.float32
BF16 = mybir.dt.bfloat16
I16 = mybir.dt.int16

QTW = 512  # q-tile width
UPB = 256  # units per batch: 32 k-chunks x 8 q-tiles
UT = UPB * BS  # 512 units total
NTILES = (UT + 2) // 3  # 171 sp tiles (last holds 2 units)

# Schraudolph affine-int exp on DVE: bf16 bits = round(s*SCALE*log2(e)*128
# + (127<<7) + shift).  shift=-8 zero-means the (1+f)->2^f interp error;
# +0.5 converts HW truncation to rounding.  |err| ~ 1.8% rms per element,
# attenuated by softmax averaging; only EXP_PAT 'D' tiles use it.
EXP_A = float(SCALE * np.log2(np.e) * 128.0)  # 23.0831
EXP_B = 127.0 * 128.0 - 8.0 + 0.5
# per-tile exp engine: 'A' = ACT exact, 'D' = DVE Schraudolph
# 50/50 alternation: buffer b=t%2 always pairs with one engine, so the two
# sp ping-pong buffers pipeline on independent engines every tile.
EXP_PAT = "AD"


def build_body(nc, tc, Qd, Kd, Vd, Od, Dd=None):
    with (
        tc.tile_pool(name="const", bufs=1) as constp,
        tc.tile_pool(name="qk", bufs=2) as qkp,
        tc.tile_pool(name="v", bufs=2) as vp,
        tc.tile_pool(name="nat", bufs=2) as natp,
        tc.tile_pool(name="sp", bufs=2, space="PSUM") as spp,
        tc.tile_pool(name="ot", bufs=1, space="PSUM") as opp,
        tc.tile_pool(name="rs", bufs=1, space="PSUM") as rsp,
        tc.tile_pool(name="et", bufs=6) as ep,
        tc.tile_pool(name="fin", bufs=6) as fp,
        tc.tile_pool(name="xst", bufs=8) as stp,
    ):
        ones32 = constp.tile([128, 32], BF16)
        nc.vector.memset(ones32[:], 1.0)
        warm = constp.tile([1, 1], F32)
        nc.scalar.activation(
            warm[:], ones32[0:1, 0:1], mybir.ActivationFunctionType.Exp, scale=1.0
        )
        # stacked identity [I64; I64] for the phase-C fold+transpose matmul
        ident2 = constp.tile([128, 64], BF16)
        nc.vector.memset(ident2[:], 0.0)
        for half in range(2):
            nc.gpsimd.affine_select(
                out=ident2[64 * half : 64 * (half + 1), :],
                in_=ident2[64 * half : 64 * (half + 1), :],
                compare_op=mybir.AluOpType.not_equal, fill=1.0, base=0,
                pattern=[[-1, 64]], channel_multiplier=1,
            )
        # 4-hot selector over the rs quarter-sum rows {0,32,64,96}
        wsel = constp.tile([128, 1], BF16)
        nc.vector.memset(wsel[:], 0.0)
        for t4 in range(4):
            nc.vector.memset(wsel[32 * t4 : 32 * t4 + 1, :], 1.0)
        identB = constp.tile([128, 128], BF16)
        nc.vector.memset(identB[:], 0.0)
        nc.gpsimd.affine_select(
            out=identB[:], in_=identB[:],
            compare_op=mybir.AluOpType.not_equal, fill=1.0, base=0,
            pattern=[[-1, 128]], channel_multiplier=1,
        )

        handles = [None] * BS  # (qt2, ktp, vq4)
        nat = [None] * BS  # (kn3, qn4)
        sp_map = {}
        et_map = {}
        ot_cur = [None]
        rs_cur = [None]
        bg = []

        # ---------------- phase A emitters ----------------
        def a_alloc(b):
            qt2 = qkp.tile([128, S], BF16, tag="qt2")
            ktp = qkp.tile([128, 2048], BF16, tag="ktp")
            vq = vp.tile([128, 16 * 2 * 64], BF16, tag="vq")
            vq4 = vq[:].rearrange("p (g h d) -> p g h d", h=2, d=64)
            handles[b] = (qt2, ktp, vq4)
            kn = natp.tile([128, 32 * 64], BF16, tag="kn")
            kn3 = kn[:].rearrange("p (c d) -> p c d", d=64)
            qn = natp.tile([128, 32 * 2 * 64], BF16, tag="qn")
            qn4 = qn[:].rearrange("p (c two d) -> p c two d", two=2, d=64)
            nat[b] = (kn3, qn4)

        def a_k_load(b, quarter):  # 8 chunks
            kn3, _ = nat[b]
            nc.gpsimd.dma_start(
                out=kn3[:, quarter * 8 : (quarter + 1) * 8, :],
                in_=Kd[b, quarter * 1024 : (quarter + 1) * 1024].rearrange(
                    "(c r) d -> r c d", r=128
                ),
            )

        def t_k(b, j, dst):  # chunk pair (2j, 2j+1) -> ktp cols [128j, 128j+128)
            kn3, _ = nat[b]
            _, ktp, _ = handles[b]
            nc.tensor.transpose(dst, kn3[:, 2 * j : 2 * j + 2, :], identB[:])
            nc.vector.tensor_copy(ktp[:, j * 128 : (j + 1) * 128], dst)

        def rx_k(b, j):  # far-margin variant on the DMA-xbar ring
            kn3, _ = nat[b]
            _, ktp, _ = handles[b]
            xst = stp.tile([128, 128], BF16, tag="xst", name="xst")
            nc.sync.dma_start(
                out=xst[:], in_=kn3[:, 2 * j : 2 * j + 2, :], transpose=True
            )
            nc.gpsimd.tensor_copy(ktp[:, j * 128 : (j + 1) * 128], xst[:])

        def rx_q(b, c):
            _, qn4 = nat[b]
            qt2, _, _ = handles[b]
            xst = stp.tile([128, 128], BF16, tag="xst", name="xst")
            nc.sync.dma_start(out=xst[:], in_=qn4[:, c, :, :], transpose=True)
            nc.gpsimd.tensor_copy(qt2[:, c * 128 : (c + 1) * 128], xst[:])

        def a_q_load(b, piece):  # 8 chunks, doubled
            _, qn4 = nat[b]
            src = Qd[b, piece * 1024 : (piece + 1) * 1024].rearrange(
                "(c r) d -> r c d", r=128
            )
            nc.gpsimd.dma_start(out=qn4[:, piece * 8 : (piece + 1) * 8, 0, :], in_=src)
            nc.gpsimd.dma_start(out=qn4[:, piece * 8 : (piece + 1) * 8, 1, :], in_=src)

        def t_q(b, c, dst):  # chunk c duplicated -> qt2 cols [128c, 128c+128)
            _, qn4 = nat[b]
            qt2, _, _ = handles[b]
            nc.tensor.transpose(dst, qn4[:, c, :, :], identB[:])
            nc.vector.tensor_copy(qt2[:, c * 128 : (c + 1) * 128], dst)

        def a_v_load(b, half):  # 8 chunk-pairs
            _, _, vq4 = handles[b]
            nc.gpsimd.dma_start(
                out=vq4[:, half * 8 : (half + 1) * 8, :, :],
                in_=Vd[b, half * 2048 : (half + 1) * 2048].rearrange(
                    "(g h r) d -> r g h d", h=2, r=128
                ),
            )

        def a_bg_items(b):
            # loads only; transposes go through the tq window queue
            items = [lambda: a_alloc(b)]
            for qtr in range(4):
                items.append(lambda q=qtr: a_k_load(b, q))
            for piece in range(4):
                items.append(lambda p=piece: a_q_load(b, p))
            for hf in range(2):
                items.append(lambda h=hf: a_v_load(b, h))
            return items

        # ---------------- phase B emitters ----------------
        def emit_S(u):
            b, r = u // UPB, u % UPB
            qi, g, h = r // 32, (r % 32) // 2, r % 2
            if u % 3 == 0:
                sp_map[u // 3] = spp.tile([128, 1536], F32, tag="sp", name="sp")
            sp = sp_map[u // 3]
            off = (u % 3) * QTW
            qt2, ktp, _ = handles[b]
            rhs = qt2[64 * h : 64 * h + 64, qi * QTW : (qi + 1) * QTW]
            nc.tensor.matmul(
                sp[:, off : off + QTW],
                ktp[64 * h : 64 * h + 64, g * 128 : (g + 1) * 128],
                rhs, start=True, stop=True, skip_group_check=True,
            )

        def emit_exp(t):
            sp = sp_map.pop(t)
            n = min(3, UT - t * 3)
            if EXP_PAT[t % len(EXP_PAT)] == "D":
                eti = ep.tile([128, 1536], I16, tag="et", name="eti")
                nc.vector.tensor_scalar(
                    out=eti[:, 0 : n * QTW], in0=sp[:, 0 : n * QTW],
                    scalar1=EXP_A, scalar2=EXP_B,
                    op0=mybir.AluOpType.mult, op1=mybir.AluOpType.add,
                )
                et = eti[:].bitcast(BF16)
            else:
                et = ep.tile([128, 1536], BF16, tag="et")
                nc.scalar.activation(
                    et[:, 0 : n * QTW], sp[:, 0 : n * QTW],
                    mybir.ActivationFunctionType.Exp, scale=SCALE,
                )
            for i in range(n):
                et_map[t * 3 + i] = (et, i * QTW)

        def emit_AV(u):  # u odd: chunk pair (2g, 2g+1) complete
            b, r = u // UPB, u % UPB
            g = (r % 32) // 2
            _, _, vq4 = handles[b]
            if g == 0:
                ot_cur[0] = opp.tile([128, QTW], F32, tag="ot", name="ot")
            ot = ot_cur[0]
            e0, o0 = et_map[u - 1]
            e1, o1 = et_map[u]
            for half in range(2):
                e, o = (e0, o0) if half == 0 else (e1, o1)
                nc.tensor.matmul(
                    ot[64 * half : 64 * (half + 1), :],
                    vq4[:, g, half, :],
                    e[:, o : o + QTW],
                    start=(g == 0), stop=(g == 15),
                    skip_group_check=True, tile_position=(0, 64 * half),
                )

        def emit_rs(u):  # (u%32)%4 == 3
            jq = (u % 32) // 4
            if jq == 0:
                rs_cur[0] = rsp.tile([128, QTW], F32, tag="rs", name="rs")
            rs = rs_cur[0]
            for t4 in range(4):
                e, o = et_map[u - 3 + t4]
                nc.tensor.matmul(
                    rs[32 * t4 : 32 * (t4 + 1), :],
                    ones32[:],
                    e[:, o : o + QTW],
                    start=(jq == 0), stop=(jq == 7),
                    skip_group_check=True, tile_position=(0, 32 * t4),
                )

        pc_pending = [None]

        def emit_phaseC_casts(b, qi):  # evacuate ot/rs to SBUF (deferred so
            # the AV/rs stop-matmuls they read are already complete: a
            # waiting op in an exp engine's queue delays the next exp).
            # osb on ACT, rsb on DVE splits the cost across both chains.
            ot, rs = ot_cur[0], rs_cur[0]
            osb = fp.tile([128, QTW], BF16, tag="osb")
            nc.vector.tensor_copy(osb[:], ot[:])
            rsb = fp.tile([128, QTW], BF16, tag="rsb")
            nc.vector.tensor_copy(rsb[:], rs[:])
            pc_pending[0] = (b, qi, osb, rsb)
            if DEBUG_DUMP and b == 0 and qi == 0:
                nc.sync.dma_start(out=Dd[:, 2560:3072], in_=osb[:])
                nc.sync.dma_start(out=Dd[:, 3072:3584], in_=rsb[:])

        tq = []  # pending (kind, batch, idx) phase-A transposes

        def emit_phaseC_fin(split=False):
            # deferred PE fold+transpose: runs a couple of units into the
            # next q tile, after the casts have completed, in the rs bank.
            # The same window also hosts up to 8 phase-A PE transposes.
            if pc_pending[0] is None:
                return
            b, qi, osb, rsb = pc_pending[0]
            pc_pending[0] = None
            if tq:
                trp = rsp.tile([128, 1024], BF16, tag="rs", name="trp")
                for s in range(4):
                    if not tq:
                        break
                    kind, tb, idx = tq.pop(0)
                    dst = trp[:, s * 128 : (s + 1) * 128]
                    if kind == "k":
                        t_k(tb, idx, dst)
                    else:
                        t_q(tb, idx, dst)
            ctp = rsp.tile([128, QTW], F32, tag="rs", name="ctp")
            rinv = fp.tile([128, 4], F32, tag="rinv")
            ob = fp.tile([128, 4 * 64], F32, tag="ob")
            # split=True (final q tile only): pipeline by column halves so
            # the DVE chain and output DMA of half 0 overlap half 1's PE.
            halves = ((0, 1), (2, 3)) if split else ((0, 1, 2, 3),)
            for jg in halves:
                for j in jg:
                    js = slice(j * 128, (j + 1) * 128)
                    nc.tensor.matmul(
                        ctp[:, j * 64 : (j + 1) * 64], osb[:, js], ident2[:],
                        start=True, stop=True, skip_group_check=True,
                    )
                    nc.tensor.matmul(
                        ctp[:, 256 + j : 257 + j], rsb[:, js], wsel[:],
                        start=True, stop=True, skip_group_check=True,
                    )
                j0, j1 = jg[0], jg[-1] + 1
                nc.vector.reciprocal(
                    rinv[:, j0:j1], ctp[:, 256 + j0 : 256 + j1]
                )
                for j in jg:
                    nc.vector.tensor_scalar_mul(
                        ob[:, j * 64 : (j + 1) * 64],
                        ctp[:, j * 64 : (j + 1) * 64],
                        rinv[:, j : j + 1],
                    )
                nc.sync.dma_start(
                    out=Od[
                        b, qi * QTW + j0 * 128 : qi * QTW + j1 * 128
                    ].rearrange("(c p) d -> p c d", p=128),
                    in_=ob[:, j0 * 64 : j1 * 64].rearrange(
                        "p (c d) -> p c d", d=64
                    ),
                )

        # ---------------- schedule ----------------
        # batch 0 head: all of K and Q piece 0 transposed on the PE into
        # the (still idle) sp banks. The full-row transposes also block
        # the PE's LDWEIGHTS pull-ahead from reading ktp/qt2 early.
        a_alloc(0)
        a_k_load(0, 0)
        a_q_load(0, 0)
        a_k_load(0, 1)
        a_k_load(0, 2)
        a_k_load(0, 3)
        a_v_load(0, 0)
        a_v_load(0, 1)
        kn3_0, qn4_0 = nat[0]
        qt2_0, ktp_0, _ = handles[0]

        def head_xpose(order, ktrv, base=0):
            for s0, (kind, idx) in enumerate(order):
                s = base + s0
                dst = ktrv[:, s * 128 : (s + 1) * 128]
                if kind == "k":
                    nc.tensor.transpose(
                        dst, kn3_0[:, 2 * idx : 2 * idx + 2, :], identB[:]
                    )
                    nc.vector.tensor_copy(
                        ktp_0[:, idx * 128 : (idx + 1) * 128], dst
                    )
                else:
                    nc.tensor.transpose(dst, qn4_0[:, idx, :, :], identB[:])
                    nc.vector.tensor_copy(
                        qt2_0[:, idx * 128 : (idx + 1) * 128], dst
                    )

        # critical transposes (tiles 0/1 need k chunks 0-15, q 0:512) in an
        # sp scratch before the S stream; the remaining 12 head transposes
        # go into the still-idle ot/rs banks right after S(0..5), so the
        # first exps start earlier and nothing queues behind exp(0).
        ktr1 = spp.tile([128, 1536], F32, tag="sp", name="ktr1")
        ktr1v = ktr1[:].bitcast(BF16)
        # exp(0) needs only tile 0 (chunks 0-2, q 0:512): emit its S
        # matmuls as soon as those 6 transposes are in the queue
        head_xpose(
            [("k", 0), ("k", 1), ("q", 0), ("q", 1), ("q", 2), ("q", 3)],
            ktr1v,
        )
        for u in range(3):  # tile 0
            emit_S(u)
        head_xpose([("k", 2), ("k", 3)], ktr1v, base=6)
        for u in range(3, 6):  # tile 1
            emit_S(u)
        head_xpose([("k", 4), ("k", 5), ("k", 6), ("k", 7)], ktr1v, base=8)
        ktr_ot = opp.tile([128, QTW], F32, tag="ot", name="ktr_ot")
        head_xpose(
            [("k", j) for j in range(8, 16)], ktr_ot[:].bitcast(BF16)
        )
        ktr_rs = rsp.tile([128, QTW], F32, tag="rs", name="ktr_rs")
        head_xpose(
            [("q", c) for c in range(4, 8)], ktr_rs[:].bitcast(BF16)
        )
        bg.append(lambda: a_q_load(0, 1))
        bg.append(lambda: a_q_load(0, 2))
        bg.append(lambda: a_q_load(0, 3))
        bg.extend([lambda c=c: rx_q(0, c) for c in range(8, 32)])
        dbg_et = [None]
        s_pend = []  # S units held so emissions are always (even,odd) pairs
        rs_pend = []  # rs units held so quads flush in adjacent twos
        cast_pend = [None]  # (b, qi) of a finished q-tile awaiting casts
        for t in range(NTILES):
            emit_exp(t)
            if DEBUG_DUMP and t == 0:
                dbg_et[0] = fp.tile([128, 1536], BF16, tag="dbg_et", name="dbg_et")
                nc.vector.tensor_copy(dbg_et[0][:], et_map[0][0][:])
            if DEBUG_DUMP and t == 30:
                qt2_0, ktp_0, _ = handles[0]
                nc.sync.dma_start(out=Dd[:, 0:512], in_=qt2_0[:, 0:512])
                nc.sync.dma_start(out=Dd[:, 512:1024], in_=ktp_0[:, 0:512])
                nc.sync.dma_start(out=Dd[:, 1024:2560], in_=dbg_et[0][:])
            # emit the whole window immediately (pair + single): completing
            # the fill ASAP shortens the sp buffer cycle, which paces the
            # kernel; a lone 512-col matmul costs the same 225ns slot.
            for u in range((t + 2) * 3, min((t + 2) * 3 + 3, UT)):
                emit_S(u)
            for u in range(t * 3, min(t * 3 + 3, UT)):
                if u % 32 == 2 and cast_pend[0] is not None:
                    emit_phaseC_casts(*cast_pend[0])
                    cast_pend[0] = None
                if u % 32 == 4:
                    emit_phaseC_fin()
                if u % 2 == 1 and u % 32 != 1:
                    if u % 32 == 3:
                        emit_AV(u - 2)
                    emit_AV(u)
                if (u % 32) % 4 == 3:
                    rs_pend.append(u)
                if len(rs_pend) >= 2 or (rs_pend and u % 32 == 31):
                    for ru in rs_pend:
                        emit_rs(ru)
                    rs_pend.clear()
                if u % 32 == 31:
                    cast_pend[0] = (u // UPB, (u % UPB) // 32)
                for uu in list(et_map):
                    if uu < u - 8:
                        del et_map[uu]
            if t == 30 and BS > 1:
                bg.extend(a_bg_items(1))
                bg.extend([lambda j=j: rx_k(1, j) for j in range(16)])
                bg.extend([lambda c=c: rx_q(1, c) for c in range(32)])
            ndrip = 3 if len(bg) > 30 else 2
            for _ in range(ndrip):
                if bg:
                    bg.pop(0)()
        assert not bg, f"{len(bg)} phase-A items left undripped"
        assert not tq, f"{len(tq)} transposes left unscheduled"
        assert not rs_pend and not s_pend
        if cast_pend[0] is not None:
            emit_phaseC_casts(*cast_pend[0])
            cast_pend[0] = None
        emit_phaseC_fin(split=True)  # final q tile


_nc_cache = None


def build_nc():
    global _nc_cache
    if _nc_cache is not None:
        return _nc_cache
    nc = bacc.Bacc(None, target_bir_lowering=False)
    Qd = nc.declare_dram_parameter("Q", [BS, S, D], F32, isOutput=False)
    Kd = nc.declare_dram_parameter("K", [BS, S, D], F32, isOutput=False)
    Vd = nc.declare_dram_parameter("V", [BS, S, D], F32, isOutput=False)
    Od = nc.declare_dram_parameter("out", [BS, S, D], F32, isOutput=True)
    Dd = None
    if DEBUG_DUMP:
        Dd = nc.declare_dram_parameter("dbg", [128, 3584], BF16, isOutput=True)
    with TileContext(nc) as tc:
        build_body(nc, tc, Qd, Kd, Vd, Od, Dd)
    nc.finalize()
    _nc_cache = nc
    return nc


def kernel(Q, K, V):
    Q = np.asarray(Q, dtype=np.float32)
    K = np.asarray(K, dtype=np.float32)
    V = np.asarray(V, dtype=np.float32)
    nc = build_nc()
    in_maps = [
        {
            "Q": np.ascontiguousarray(Q[i * BS : (i + 1) * BS]),
            "K": np.ascontiguousarray(K[i * BS : (i + 1) * BS]),
            "V": np.ascontiguousarray(V[i * BS : (i + 1) * BS]),
        }
        for i in range(N_CORES)
    ]
    res = run_bass_kernel_spmd(nc, in_maps, core_ids=list(range(N_CORES)))
    return np.concatenate([res.results[i]["out"] for i in range(N_CORES)], axis=0)

